# revision 16
# baseline (speedup 1.0000x reference)
"""Trainium2 Bass kernel for 16-head causal attention (B=2, L=2048, D=1024).

Sharding (8 NeuronCores, one chip):
  - Head tensor-parallel: core c computes heads {2c, 2c+1} for BOTH batches.
    QKV projections are computed in transposed (d-major) layout directly:
    Q^T = Wq_pair @ x^T (x^T and W^T are host-prepped), so the S^T = K^T x Q
    matmul needs no on-device activation transposes. V is computed kv-major
    directly (x as the stationary operand), so no transposes there either.
  - Attention: S^T tiles [128 kv, 512 q], additive causal mask on diagonal
    blocks, un-stabilized exp on ScalarE (scale=1/8 folded into ACT), P*V with
    a ones-augmented V (softmax denominators fall out of the matmul).
  - One 8-way AllToAll redistributes ctx^T from head-sharded to
    (batch, L/4-slice)-sharded; each core then computes
    out^T = Wo @ ctx_full^T for its 512-row output slice.
  - Host: transpose/concat per-core out^T slices into the full output.

Precision: fp32/f32r matmuls are pathologically slow on this target
(~55us/instruction), so all matmuls run in bf16 with fp32 PSUM accumulation,
using hi+lo split operands (bf16x3: A@B ~= Ah@Bh + Ah@Bl + Al@Bh) everywhere
except the P-side of the P*V matmul (attention-weight rounding averages out).
Measured ~7.5e-4 scale-relative output error vs the fp32 reference.
"""

import numpy as np
import ml_dtypes

import concourse.bass as bass
import concourse.mybir as mybir
import concourse.tile as tile
from concourse import bacc
from concourse.bass_utils import run_bass_kernel_spmd

F32 = mybir.dt.float32
BF16 = mybir.dt.bfloat16

B, L, D = 2, 2048, 1024
NCORES = 8
QC = 512   # q-chunk width
KB = 128   # kv-block width
NEG = -3.0e4  # additive mask; exp(scale*NEG) == 0 exactly in fp32

_CACHE: dict = {}


def _build_program(n_iters: int = 1, debug: bool = False, use_collective: bool = True):
    """Build the SPMD Bass program (same on all cores; per-core data differs).

    n_iters > 1 repeats the whole compute body for wall-clock timing.
    debug=True enables named-tensor access for the simulator.
    use_collective=False skips the A2A (timing experiments only; wrong output).
    """
    nc = bacc.Bacc(debug=debug)

    par = {}
    for nm, shape, dt in (
        ("xh", [B, D, L], BF16),
        ("xl", [B, D, L], BF16),
        ("wqh", [D, 128], BF16), ("wql", [D, 128], BF16),
        ("wkh", [D, 128], BF16), ("wkl", [D, 128], BF16),
        ("wvh", [D, 128], BF16), ("wvl", [D, 128], BF16),
        ("woh", [D, D], BF16), ("wol", [D, D], BF16),
        ("maskadd", [4, 128, QC], F32),
    ):
        par[nm] = nc.declare_dram_parameter(nm, shape, dt, isOutput=False)
    outT = nc.declare_dram_parameter("outT", [D, QC], F32, isOutput=True)

    # A2A payload: shard j rows [256j:256j+256] = [ctx_hi block j; ctx_lo block j]
    a2a_in = nc.dram_tensor("a2a_in", [NCORES * 256, QC], BF16)
    a2a_out = nc.dram_tensor("a2a_out", [NCORES * 256, QC], BF16)
    groups = [list(range(NCORES))]

    with tile.TileContext(nc) as tc:
        for _ in range(n_iters):
            _emit_iteration(nc, tc, par, outT, a2a_in, a2a_out, groups,
                            use_collective)

    nc.compile()
    return nc


def _emit_iteration(nc, tc, par, outT, a2a_in, a2a_out, groups,
                    use_collective=True):
    with (
        tc.tile_pool(name="const", bufs=1) as const_p,
        tc.tile_pool(name="w", bufs=1) as w_p,
        tc.tile_pool(name="qk", bufs=2) as qk_p,
        tc.tile_pool(name="vt", bufs=64) as v_p,
        tc.tile_pool(name="ctx", bufs=2) as ctx_p,
        tc.tile_pool(name="msk", bufs=4) as msk_p,
        tc.tile_pool(name="p", bufs=4) as p_p,
        tc.tile_pool(name="r", bufs=2) as r_p,
        tc.tile_pool(name="bs", bufs=2) as bs_p,
        tc.tile_pool(name="cf", bufs=2) as cf_p,
        tc.tile_pool(name="ps", bufs=2, space="PSUM") as ps_p,
        tc.tile_pool(name="st", bufs=4, space="PSUM") as st_p,
        tc.tile_pool(name="cps", bufs=2, space="PSUM") as cps_p,
    ):
        # ---- constants ----
        ones_f = const_p.tile([128, 1], F32)
        nc.gpsimd.memset(ones_f[:], 1.0)
        ones64 = const_p.tile([1, 64], BF16)
        nc.vector.tensor_copy(ones64[:], ones_f[0:1, 0:1].to_broadcast([1, 64]))

        mask_sb = []
        for t in range(4):
            m = msk_p.tile([128, QC], F32, tag="mask")
            nc.sync.dma_start(m[:], par["maskadd"][t])
            mask_sb.append(m)

        # ---- weights (QKV, hi+lo) ----
        w_sb = {}
        for name in ("wqh", "wql", "wkh", "wkl", "wvh", "wvl"):
            w = w_p.tile([128, 8, 128], BF16, tag=name, name=name)
            nc.sync.dma_start(w[:], par[name].rearrange("(a p) m -> p a m", p=128))
            w_sb[name] = w

        ctx_sb = []  # per batch (hi, lo) [128, 2048] bf16
        with tc.tile_pool(name="xt", bufs=16) as xt_p:
            for b in range(B):
                # ---- load x^T (hi+lo) for this batch ----
                xh, xl = [], []
                for a in range(8):
                    th = xt_p.tile([128, L], BF16, tag="xt", name=f"xh{a}")
                    nc.sync.dma_start(th[:], par["xh"][b, 128 * a : 128 * (a + 1), :])
                    xh.append(th)
                    tl = xt_p.tile([128, L], BF16, tag="xt", name=f"xl{a}")
                    nc.sync.dma_start(tl[:], par["xl"][b, 128 * a : 128 * (a + 1), :])
                    xl.append(tl)

                # ---- Q/K projections (d-major, split3) ----
                def proj_dmajor(wh, wl, dsth, dstl):
                    for n in range(4):
                        ns = slice(QC * n, QC * (n + 1))
                        ps = ps_p.tile([128, QC], F32, tag="ps", name="pps")
                        for a in range(8):
                            for i, (wop, xop) in enumerate(
                                ((wh, xh[a]), (wh, xl[a]), (wl, xh[a]))
                            ):
                                nc.tensor.matmul(
                                    ps[:],
                                    lhsT=wop[:, a, :],
                                    rhs=xop[:, ns],
                                    start=(a == 0 and i == 0),
                                    stop=(a == 7 and i == 2),
                                )
                        nc.vector.tensor_copy(dsth[:, ns], ps[:])
                        nc.vector.tensor_sub(dstl[:, ns], ps[:], dsth[:, ns])

                qTh = qk_p.tile([128, L], BF16, tag="qTh", name="qTh")
                qTl = qk_p.tile([128, L], BF16, tag="qTl", name="qTl")
                kTh = qk_p.tile([128, L], BF16, tag="kTh", name="kTh")
                kTl = qk_p.tile([128, L], BF16, tag="kTl", name="kTl")
                proj_dmajor(w_sb["wqh"], w_sb["wql"], qTh, qTl)
                proj_dmajor(w_sb["wkh"], w_sb["wkl"], kTh, kTl)

                # ---- V projection (kv-major, split3), ones column appended ----
                vth_tiles, vtl_tiles = [], []
                for kb in range(16):
                    ks = slice(KB * kb, KB * (kb + 1))
                    ps = ps_p.tile([128, 128], F32, tag="ps", name="vps")
                    for a in range(8):
                        for i, (xop, wop) in enumerate(
                            (
                                (xh[a], w_sb["wvh"]),
                                (xl[a], w_sb["wvh"]),
                                (xh[a], w_sb["wvl"]),
                            )
                        ):
                            nc.tensor.matmul(
                                ps[:],
                                lhsT=xop[:, ks],
                                rhs=wop[:, a, :],
                                start=(a == 0 and i == 0),
                                stop=(a == 7 and i == 2),
                            )
                    vth = v_p.tile([128, 130], BF16, tag="vth", name="vth")
                    vtl = v_p.tile([128, 130], BF16, tag="vtl", name="vtl")
                    for a in range(2):
                        hs = slice(65 * a, 65 * a + 64)
                        ds = slice(64 * a, 64 * (a + 1))
                        nc.vector.tensor_copy(vth[:, hs], ps[:, ds])
                        nc.vector.tensor_sub(vtl[:, hs], ps[:, ds], vth[:, hs])
                        # ones column: sums come from hi only; lo contributes 0
                        nc.vector.tensor_copy(
                            vth[:, 65 * a + 64 : 65 * a + 65], ones_f[:]
                        )
                        nc.gpsimd.memset(vtl[:, 65 * a + 64 : 65 * a + 65], 0.0)
                    vth_tiles.append(vth)
                    vtl_tiles.append(vtl)

                # ---- attention ----
                ctxh = ctx_p.tile([128, L], BF16, tag="ctxh", name="ctxh")
                ctxl = ctx_p.tile([128, L], BF16, tag="ctxl", name="ctxl")
                ctx_sb.append((ctxh, ctxl))
                for jc in range(4):
                    qs = slice(QC * jc, QC * (jc + 1))
                    cps = [
                        cps_p.tile([65, QC], F32, tag="cps", name=f"cps{a}")
                        for a in range(2)
                    ]
                    nkb = 4 * jc + 4
                    for kb in range(nkb):
                        ks = slice(KB * kb, KB * (kb + 1))
                        p_t = []
                        for a in range(2):
                            da = slice(64 * a, 64 * (a + 1))
                            st = st_p.tile([128, QC], F32, tag="st", name="st")
                            for i, (kop, qop) in enumerate(
                                ((kTh, qTh), (kTh, qTl), (kTl, qTh))
                            ):
                                nc.tensor.matmul(
                                    st[:],
                                    lhsT=kop[da, ks],
                                    rhs=qop[da, qs],
                                    start=(i == 0),
                                    stop=(i == 2),
                                    tile_position=(64 * a, 0),
                                )
                            if kb >= 4 * jc:
                                w = 128 * (kb - 4 * jc + 1)
                                nc.vector.tensor_add(
                                    st[:, :w], st[:, :w], mask_sb[kb - 4 * jc][:, :w]
                                )
                            p = p_p.tile([128, QC], BF16, tag="p", name="p")
                            nc.scalar.activation(
                                p[:], st[:], mybir.ActivationFunctionType.Exp,
                                scale=0.125,
                            )
                            p_t.append(p)
                        for a in range(2):
                            hs = slice(65 * a, 65 * a + 65)
                            nc.tensor.matmul(
                                cps[a][:], lhsT=vth_tiles[kb][:, hs], rhs=p_t[a][:],
                                start=(kb == 0), stop=False,
                            )
                            nc.tensor.matmul(
                                cps[a][:], lhsT=vtl_tiles[kb][:, hs], rhs=p_t[a][:],
                                start=False, stop=(kb == nkb - 1),
                            )
                    # normalize: ctx rows for each head *= 1/denominator (split)
                    for a in range(2):
                        rh = r_p.tile([1, QC], BF16, tag="rh", name="rh")
                        rl = r_p.tile([1, QC], BF16, tag="rl", name="rl")
                        rf = r_p.tile([1, QC], F32, tag="rf", name="rf")
                        nc.vector.reciprocal(rf[:], cps[a][64:65, :])
                        nc.vector.tensor_copy(rh[:], rf[:])
                        nc.vector.tensor_sub(rl[:], rf[:], rh[:])
                        bc = ps_p.tile([64, QC], F32, tag="ps", name="bps")
                        nc.tensor.matmul(bc[:], lhsT=ones64[:], rhs=rh[:],
                                         start=True, stop=False)
                        nc.tensor.matmul(bc[:], lhsT=ones64[:], rhs=rl[:],
                                         start=False, stop=True)
                        bs = bs_p.tile([64, QC], F32, tag="bs", name="bs")
                        nc.vector.tensor_copy(bs[:], bc[:])
                        cf = cf_p.tile([64, QC], F32, tag="cf", name="cf")
                        nc.vector.tensor_mul(cf[:], cps[a][0:64, :], bs[:])
                        ch = cf_p.tile([64, QC], BF16, tag="ch", name="ch")
                        cl = cf_p.tile([64, QC], BF16, tag="cl", name="cl")
                        nc.vector.tensor_copy(ch[:], cf[:])
                        nc.vector.tensor_sub(cl[:], cf[:], ch[:])
                        da = slice(64 * a, 64 * (a + 1))
                        nc.vector.tensor_copy(ctxh[da, qs], ch[:])
                        nc.vector.tensor_copy(ctxl[da, qs], cl[:])

        # ---- A2A staging: 4 DMAs (batch x hi/lo) ----
        # a2a rows: 256*j + 128*h + p  (j = 4b + t; h: 0=hi, 1=lo)
        a2a_in_v = a2a_in.rearrange("(j h p) n -> h p j n", h=2, p=128)
        for b in range(B):
            for h, t in ((0, ctx_sb[b][0]), (1, ctx_sb[b][1])):
                nc.sync.dma_start(
                    a2a_in_v[h, :, 4 * b : 4 * (b + 1), :],
                    t[:].rearrange("p (t n) -> p t n", n=QC),
                )
        if use_collective:
            nc.gpsimd.collective_compute(
                "AllToAll",
                mybir.AluOpType.bypass,
                replica_groups=groups,
                ins=[a2a_in[:]],
                outs=[a2a_out[:]],
            )
        else:
            a2a_out = a2a_in

        # ---- out projection: out^T = Wo @ ctx_full^T (split3) ----
        with (
            tc.tile_pool(name="wo", bufs=1) as wo_p,
            tc.tile_pool(name="ao", bufs=16) as ao_p,
            tc.tile_pool(name="os", bufs=4) as os_p,
        ):
            woh = wo_p.tile([128, 8, D], BF16, tag="woh", name="woh")
            nc.sync.dma_start(woh[:], par["woh"].rearrange("(a p) m -> p a m", p=128))
            wol = wo_p.tile([128, 8, D], BF16, tag="wol", name="wol")
            nc.sync.dma_start(wol[:], par["wol"].rearrange("(a p) m -> p a m", p=128))
            aoh, aol = [], []
            a2a_out_v = a2a_out.rearrange("(j h p) n -> h j p n", h=2, p=128)
            for a in range(8):
                th = ao_p.tile([128, QC], BF16, tag="ao", name=f"aoh{a}")
                nc.sync.dma_start(th[:], a2a_out_v[0, a])
                aoh.append(th)
                tl = ao_p.tile([128, QC], BF16, tag="ao", name=f"aol{a}")
                nc.sync.dma_start(tl[:], a2a_out_v[1, a])
                aol.append(tl)
            for ob in range(8):
                os_ = slice(128 * ob, 128 * (ob + 1))
                ps = ps_p.tile([128, QC], F32, tag="ps", name="ops")
                for a in range(8):
                    for i, (wop, aop) in enumerate(
                        ((woh, aoh[a]), (woh, aol[a]), (wol, aoh[a]))
                    ):
                        nc.tensor.matmul(
                            ps[:],
                            lhsT=wop[:, a, os_],
                            rhs=aop[:],
                            start=(a == 0 and i == 0),
                            stop=(a == 7 and i == 2),
                        )
                o_sb = os_p.tile([128, QC], F32, tag="os", name="osb")
                nc.vector.tensor_copy(o_sb[:], ps[:])
                nc.sync.dma_start(outT[os_, :], o_sb[:])


def _split(x):
    hi = np.asarray(x, np.float32).astype(ml_dtypes.bfloat16)
    lo = (np.asarray(x, np.float32) - hi.astype(np.float32)).astype(ml_dtypes.bfloat16)
    return np.ascontiguousarray(hi), np.ascontiguousarray(lo)


def _prep_in_maps(x, mask, Wq, Wk, Wv, Wo):
    x = np.asarray(x, dtype=np.float32)
    mask = np.asarray(mask)

    xT = x.transpose(0, 2, 1)  # [2, 1024, 2048]
    xh, xl = _split(xT)
    woh, wol = _split(np.asarray(Wo, np.float32).T)

    m0 = mask[0]
    maskadd = np.stack(
        [
            np.where(m0[0:QC, KB * t : KB * (t + 1)].T, 0.0, NEG).astype(np.float32)
            for t in range(4)
        ]
    )

    in_maps = []
    for c in range(NCORES):
        rows = slice(128 * c, 128 * (c + 1))
        wqh, wql = _split(np.asarray(Wq, np.float32)[rows, :].T)
        wkh, wkl = _split(np.asarray(Wk, np.float32)[rows, :].T)
        wvh, wvl = _split(np.asarray(Wv, np.float32)[rows, :].T)
        in_maps.append(
            {
                "xh": xh, "xl": xl,
                "wqh": wqh, "wql": wql,
                "wkh": wkh, "wkl": wkl,
                "wvh": wvh, "wvl": wvl,
                "woh": woh, "wol": wol,
                "maskadd": maskadd,
            }
        )
    return in_maps


def _assemble(results):
    out = np.empty((B, L, D), np.float32)
    for c in range(NCORES):
        outT = results[c]["outT"]
        out[c // 4, QC * (c % 4) : QC * (c % 4 + 1), :] = outT.T
    return out


def get_program(n_iters: int = 1, debug: bool = False, use_collective: bool = True):
    key = ("prog", n_iters, debug, use_collective)
    if key not in _CACHE:
        _CACHE[key] = _build_program(n_iters, debug=debug,
                                     use_collective=use_collective)
    return _CACHE[key]


def kernel(x, mask, Wq, Wk, Wv, Wo):
    nc = get_program()
    in_maps = _prep_in_maps(x, mask, Wq, Wk, Wv, Wo)
    res = run_bass_kernel_spmd(nc, in_maps, core_ids=list(range(NCORES)))
    return _assemble(res.results)


# revision 17
# speedup vs baseline: 800.3961x; 800.3961x over previous
"""Trainium2 Bass kernel for 16-head causal attention (B=2, L=2048, D=1024).

Sharding (8 NeuronCores, one chip):
  - Head tensor-parallel: core c computes heads {2c, 2c+1} for BOTH batches.
    QKV projections are computed in transposed (d-major) layout directly:
    Q^T = Wq_pair @ x^T (x^T and W^T are host-prepped), so the S^T = K^T x Q
    matmul needs no on-device activation transposes. V is computed kv-major
    directly (x as the stationary operand), so no transposes there either.
  - Attention: S^T tiles [128 kv, 512 q], additive causal mask on diagonal
    blocks, un-stabilized exp on ScalarE (scale=1/8 folded into ACT), P*V with
    a ones-augmented V (softmax denominators fall out of the matmul).
  - One 8-way AllToAll redistributes ctx^T from head-sharded to
    (batch, L/4-slice)-sharded; each core then computes
    out^T = Wo @ ctx_full^T for its 512-row output slice.
  - Host: transpose/concat per-core out^T slices into the full output.

Precision: fp32/f32r matmuls are pathologically slow on this target
(~55us/instruction), so all matmuls run in bf16 with fp32 PSUM accumulation,
using hi+lo split operands (bf16x3: A@B ~= Ah@Bh + Ah@Bl + Al@Bh) everywhere
except the P-side of the P*V matmul (attention-weight rounding averages out).
Measured ~7.5e-4 scale-relative output error vs the fp32 reference.
"""

import numpy as np
import ml_dtypes

import concourse.bass as bass
import concourse.mybir as mybir
import concourse.tile as tile
from concourse import bacc
from concourse.bass_utils import run_bass_kernel_spmd

F32 = mybir.dt.float32
BF16 = mybir.dt.bfloat16

B, L, D = 2, 2048, 1024
NCORES = 8
QC = 512   # q-chunk width
KB = 128   # kv-block width
NEG = -3.0e4  # additive mask; exp(scale*NEG) == 0 exactly in fp32

_CACHE: dict = {}


def _build_program(n_iters: int = 1, debug: bool = False, use_collective: bool = True,
                   stages: tuple = ("proj", "v", "attn", "out")):
    """Build the SPMD Bass program (same on all cores; per-core data differs).

    n_iters > 1 repeats the whole compute body for wall-clock timing.
    debug=True enables named-tensor access for the simulator.
    use_collective=False skips the A2A (timing experiments only; wrong output).
    """
    nc = bacc.Bacc(debug=debug)

    par = {}
    for nm, shape, dt in (
        ("xh", [B, D, L], BF16),
        ("xl", [B, D, L], BF16),
        ("wqh", [D, 128], BF16), ("wql", [D, 128], BF16),
        ("wkh", [D, 128], BF16), ("wkl", [D, 128], BF16),
        ("wvh", [D, 128], BF16), ("wvl", [D, 128], BF16),
        ("woh", [D, D], BF16), ("wol", [D, D], BF16),
        ("maskadd", [4, 128, QC], F32),
    ):
        par[nm] = nc.declare_dram_parameter(nm, shape, dt, isOutput=False)
    outT = nc.declare_dram_parameter("outT", [D, QC], F32, isOutput=True)

    # A2A payload: shard j rows [256j:256j+256] = [ctx_hi block j; ctx_lo block j]
    a2a_in = nc.dram_tensor("a2a_in", [NCORES * 256, QC], BF16)
    a2a_out = nc.dram_tensor("a2a_out", [NCORES * 256, QC], BF16)
    groups = [list(range(NCORES))]

    with tile.TileContext(nc) as tc:
        for _ in range(n_iters):
            _emit_iteration(nc, tc, par, outT, a2a_in, a2a_out, groups,
                            use_collective, stages)

    nc.compile()
    return nc


def _emit_iteration(nc, tc, par, outT, a2a_in, a2a_out, groups,
                    use_collective=True, stages=("proj", "v", "attn", "out")):
    with (
        tc.tile_pool(name="const", bufs=1) as const_p,
        tc.tile_pool(name="w", bufs=1) as w_p,
        tc.tile_pool(name="qk", bufs=2) as qk_p,
        tc.tile_pool(name="vt", bufs=64) as v_p,
        tc.tile_pool(name="ctx", bufs=2) as ctx_p,
        tc.tile_pool(name="msk", bufs=4) as msk_p,
        tc.tile_pool(name="p", bufs=4) as p_p,
        tc.tile_pool(name="r", bufs=2) as r_p,
        tc.tile_pool(name="bs", bufs=2) as bs_p,
        tc.tile_pool(name="cf", bufs=2) as cf_p,
        tc.tile_pool(name="ps", bufs=2, space="PSUM") as ps_p,
        tc.tile_pool(name="st", bufs=4, space="PSUM") as st_p,
        tc.tile_pool(name="cps", bufs=2, space="PSUM") as cps_p,
    ):
        # ---- constants ----
        ones_f = const_p.tile([128, 1], F32)
        nc.gpsimd.memset(ones_f[:], 1.0)
        ones64 = const_p.tile([1, 64], BF16)
        nc.vector.tensor_copy(ones64[:], ones_f[0:1, 0:1].to_broadcast([1, 64]))

        mask_sb = []
        for t in range(4):
            m = msk_p.tile([128, QC], F32, tag="mask")
            nc.sync.dma_start(m[:], par["maskadd"][t])
            mask_sb.append(m)

        # ---- weights (QKV, hi+lo) ----
        w_sb = {}
        for name in ("wqh", "wql", "wkh", "wkl", "wvh", "wvl"):
            w = w_p.tile([128, 8, 128], BF16, tag=name, name=name)
            nc.sync.dma_start(w[:], par[name].rearrange("(a p) m -> p a m", p=128))
            w_sb[name] = w

        ctx_sb = []  # per batch (hi, lo) [128, 2048] bf16
        with tc.tile_pool(name="xt", bufs=16) as xt_p:
            for b in range(B):
                # ---- load x^T (hi+lo) for this batch ----
                xh, xl = [], []
                for a in range(8):
                    th = xt_p.tile([128, L], BF16, tag="xt", name=f"xh{a}")
                    nc.sync.dma_start(th[:], par["xh"][b, 128 * a : 128 * (a + 1), :])
                    xh.append(th)
                    tl = xt_p.tile([128, L], BF16, tag="xt", name=f"xl{a}")
                    nc.sync.dma_start(tl[:], par["xl"][b, 128 * a : 128 * (a + 1), :])
                    xl.append(tl)

                # ---- Q/K projections (d-major, split3) ----
                def proj_dmajor(wh, wl, dsth, dstl):
                    for n in range(4):
                        ns = slice(QC * n, QC * (n + 1))
                        ps = ps_p.tile([128, QC], F32, tag="ps", name="pps")
                        for a in range(8):
                            for i, (wop, xop) in enumerate(
                                ((wh, xh[a]), (wh, xl[a]), (wl, xh[a]))
                            ):
                                nc.tensor.matmul(
                                    ps[:],
                                    lhsT=wop[:, a, :],
                                    rhs=xop[:, ns],
                                    start=(a == 0 and i == 0),
                                    stop=(a == 7 and i == 2),
                                )
                        nc.vector.tensor_copy(dsth[:, ns], ps[:])
                        nc.vector.tensor_sub(dstl[:, ns], ps[:], dsth[:, ns])

                qTh = qk_p.tile([128, L], BF16, tag="qTh", name="qTh")
                qTl = qk_p.tile([128, L], BF16, tag="qTl", name="qTl")
                kTh = qk_p.tile([128, L], BF16, tag="kTh", name="kTh")
                kTl = qk_p.tile([128, L], BF16, tag="kTl", name="kTl")
                if "proj" in stages:
                    proj_dmajor(w_sb["wqh"], w_sb["wql"], qTh, qTl)
                    proj_dmajor(w_sb["wkh"], w_sb["wkl"], kTh, kTl)

                # ---- V projection (kv-major, split3), ones column appended ----
                vth_tiles, vtl_tiles = [], []
                for kb in range(16 if "v" in stages else 0):
                    ks = slice(KB * kb, KB * (kb + 1))
                    ps = ps_p.tile([128, 128], F32, tag="ps", name="vps")
                    for a in range(8):
                        for i, (xop, wop) in enumerate(
                            (
                                (xh[a], w_sb["wvh"]),
                                (xl[a], w_sb["wvh"]),
                                (xh[a], w_sb["wvl"]),
                            )
                        ):
                            nc.tensor.matmul(
                                ps[:],
                                lhsT=xop[:, ks],
                                rhs=wop[:, a, :],
                                start=(a == 0 and i == 0),
                                stop=(a == 7 and i == 2),
                            )
                    vth = v_p.tile([128, 130], BF16, tag="vth", name="vth")
                    vtl = v_p.tile([128, 130], BF16, tag="vtl", name="vtl")
                    for a in range(2):
                        hs = slice(65 * a, 65 * a + 64)
                        ds = slice(64 * a, 64 * (a + 1))
                        nc.vector.tensor_copy(vth[:, hs], ps[:, ds])
                        nc.vector.tensor_sub(vtl[:, hs], ps[:, ds], vth[:, hs])
                        # ones column: sums come from hi only; lo contributes 0
                        nc.vector.tensor_copy(
                            vth[:, 65 * a + 64 : 65 * a + 65], ones_f[:]
                        )
                        nc.gpsimd.memset(vtl[:, 65 * a + 64 : 65 * a + 65], 0.0)
                    vth_tiles.append(vth)
                    vtl_tiles.append(vtl)

                # ---- attention ----
                ctxh = ctx_p.tile([128, L], BF16, tag="ctxh", name="ctxh")
                ctxl = ctx_p.tile([128, L], BF16, tag="ctxl", name="ctxl")
                ctx_sb.append((ctxh, ctxl))
                for jc in range(4 if "attn" in stages else 0):
                    qs = slice(QC * jc, QC * (jc + 1))
                    cps = [
                        cps_p.tile([65, QC], F32, tag="cps", name=f"cps{a}")
                        for a in range(2)
                    ]
                    nkb = 4 * jc + 4
                    for kb in range(nkb):
                        ks = slice(KB * kb, KB * (kb + 1))
                        p_t = []
                        for a in range(2):
                            da = slice(64 * a, 64 * (a + 1))
                            st = st_p.tile([128, QC], F32, tag="st", name="st")
                            for i, (kop, qop) in enumerate(
                                ((kTh, qTh), (kTh, qTl), (kTl, qTh))
                            ):
                                nc.tensor.matmul(
                                    st[:],
                                    lhsT=kop[da, ks],
                                    rhs=qop[da, qs],
                                    start=(i == 0),
                                    stop=(i == 2),
                                    tile_position=(64 * a, 0),
                                )
                            if kb >= 4 * jc:
                                w = 128 * (kb - 4 * jc + 1)
                                nc.vector.tensor_add(
                                    st[:, :w], st[:, :w], mask_sb[kb - 4 * jc][:, :w]
                                )
                            p = p_p.tile([128, QC], BF16, tag="p", name="p")
                            nc.scalar.activation(
                                p[:], st[:], mybir.ActivationFunctionType.Exp,
                                scale=0.125,
                            )
                            p_t.append(p)
                        for a in range(2):
                            hs = slice(65 * a, 65 * a + 65)
                            nc.tensor.matmul(
                                cps[a][:], lhsT=vth_tiles[kb][:, hs], rhs=p_t[a][:],
                                start=(kb == 0), stop=False,
                            )
                            nc.tensor.matmul(
                                cps[a][:], lhsT=vtl_tiles[kb][:, hs], rhs=p_t[a][:],
                                start=False, stop=(kb == nkb - 1),
                            )
                    # normalize: ctx rows for each head *= 1/denominator (split)
                    for a in range(2):
                        rh = r_p.tile([1, QC], BF16, tag="rh", name="rh")
                        rl = r_p.tile([1, QC], BF16, tag="rl", name="rl")
                        rf = r_p.tile([1, QC], F32, tag="rf", name="rf")
                        nc.vector.reciprocal(rf[:], cps[a][64:65, :])
                        nc.vector.tensor_copy(rh[:], rf[:])
                        nc.vector.tensor_sub(rl[:], rf[:], rh[:])
                        bc = ps_p.tile([64, QC], F32, tag="ps", name="bps")
                        nc.tensor.matmul(bc[:], lhsT=ones64[:], rhs=rh[:],
                                         start=True, stop=False)
                        nc.tensor.matmul(bc[:], lhsT=ones64[:], rhs=rl[:],
                                         start=False, stop=True)
                        bs = bs_p.tile([64, QC], F32, tag="bs", name="bs")
                        nc.vector.tensor_copy(bs[:], bc[:])
                        cf = cf_p.tile([64, QC], F32, tag="cf", name="cf")
                        nc.vector.tensor_mul(cf[:], cps[a][0:64, :], bs[:])
                        ch = cf_p.tile([64, QC], BF16, tag="ch", name="ch")
                        cl = cf_p.tile([64, QC], BF16, tag="cl", name="cl")
                        nc.vector.tensor_copy(ch[:], cf[:])
                        nc.vector.tensor_sub(cl[:], cf[:], ch[:])
                        da = slice(64 * a, 64 * (a + 1))
                        nc.vector.tensor_copy(ctxh[da, qs], ch[:])
                        nc.vector.tensor_copy(ctxl[da, qs], cl[:])

        # ---- A2A staging: 4 DMAs (batch x hi/lo) ----
        # a2a rows: 256*j + 128*h + p  (j = 4b + t; h: 0=hi, 1=lo)
        if "attn" not in stages or "out" not in stages:
            # timing-only variant: write something to outT and skip the rest
            with tc.tile_pool(name="dummy", bufs=1) as d_p:
                dt_ = d_p.tile([128, QC], F32, tag="d", name="dummy_t")
                nc.vector.tensor_copy(dt_[:], mask_sb[0][:])
                for ob in range(8):
                    nc.sync.dma_start(outT[128 * ob : 128 * (ob + 1), :], dt_[:])
            return
        a2a_in_v = a2a_in.rearrange("(j h p) n -> h p j n", h=2, p=128)
        for b in range(B):
            for h, t in ((0, ctx_sb[b][0]), (1, ctx_sb[b][1])):
                nc.sync.dma_start(
                    a2a_in_v[h, :, 4 * b : 4 * (b + 1), :],
                    t[:].rearrange("p (t n) -> p t n", n=QC),
                )
        if use_collective:
            nc.gpsimd.collective_compute(
                "AllToAll",
                mybir.AluOpType.bypass,
                replica_groups=groups,
                ins=[a2a_in[:]],
                outs=[a2a_out[:]],
            )
        else:
            a2a_out = a2a_in

        # ---- out projection: out^T = Wo @ ctx_full^T (split3) ----
        with (
            tc.tile_pool(name="wo", bufs=1) as wo_p,
            tc.tile_pool(name="ao", bufs=16) as ao_p,
            tc.tile_pool(name="os", bufs=4) as os_p,
        ):
            woh = wo_p.tile([128, 8, D], BF16, tag="woh", name="woh")
            nc.sync.dma_start(woh[:], par["woh"].rearrange("(a p) m -> p a m", p=128))
            wol = wo_p.tile([128, 8, D], BF16, tag="wol", name="wol")
            nc.sync.dma_start(wol[:], par["wol"].rearrange("(a p) m -> p a m", p=128))
            aoh, aol = [], []
            a2a_out_v = a2a_out.rearrange("(j h p) n -> h j p n", h=2, p=128)
            for a in range(8):
                th = ao_p.tile([128, QC], BF16, tag="ao", name=f"aoh{a}")
                nc.sync.dma_start(th[:], a2a_out_v[0, a])
                aoh.append(th)
                tl = ao_p.tile([128, QC], BF16, tag="ao", name=f"aol{a}")
                nc.sync.dma_start(tl[:], a2a_out_v[1, a])
                aol.append(tl)
            for ob in range(8):
                os_ = slice(128 * ob, 128 * (ob + 1))
                ps = ps_p.tile([128, QC], F32, tag="ps", name="ops")
                for a in range(8):
                    for i, (wop, aop) in enumerate(
                        ((woh, aoh[a]), (woh, aol[a]), (wol, aoh[a]))
                    ):
                        nc.tensor.matmul(
                            ps[:],
                            lhsT=wop[:, a, os_],
                            rhs=aop[:],
                            start=(a == 0 and i == 0),
                            stop=(a == 7 and i == 2),
                        )
                o_sb = os_p.tile([128, QC], F32, tag="os", name="osb")
                nc.vector.tensor_copy(o_sb[:], ps[:])
                nc.sync.dma_start(outT[os_, :], o_sb[:])


def _split(x):
    hi = np.asarray(x, np.float32).astype(ml_dtypes.bfloat16)
    lo = (np.asarray(x, np.float32) - hi.astype(np.float32)).astype(ml_dtypes.bfloat16)
    return np.ascontiguousarray(hi), np.ascontiguousarray(lo)


def _prep_in_maps(x, mask, Wq, Wk, Wv, Wo):
    x = np.asarray(x, dtype=np.float32)
    mask = np.asarray(mask)

    xT = x.transpose(0, 2, 1)  # [2, 1024, 2048]
    xh, xl = _split(xT)
    woh, wol = _split(np.asarray(Wo, np.float32).T)

    m0 = mask[0]
    maskadd = np.stack(
        [
            np.where(m0[0:QC, KB * t : KB * (t + 1)].T, 0.0, NEG).astype(np.float32)
            for t in range(4)
        ]
    )

    in_maps = []
    for c in range(NCORES):
        rows = slice(128 * c, 128 * (c + 1))
        wqh, wql = _split(np.asarray(Wq, np.float32)[rows, :].T)
        wkh, wkl = _split(np.asarray(Wk, np.float32)[rows, :].T)
        wvh, wvl = _split(np.asarray(Wv, np.float32)[rows, :].T)
        in_maps.append(
            {
                "xh": xh, "xl": xl,
                "wqh": wqh, "wql": wql,
                "wkh": wkh, "wkl": wkl,
                "wvh": wvh, "wvl": wvl,
                "woh": woh, "wol": wol,
                "maskadd": maskadd,
            }
        )
    return in_maps


def _assemble(results):
    out = np.empty((B, L, D), np.float32)
    for c in range(NCORES):
        outT = results[c]["outT"]
        out[c // 4, QC * (c % 4) : QC * (c % 4 + 1), :] = outT.T
    return out


def get_program(n_iters: int = 1, debug: bool = False, use_collective: bool = True,
                stages: tuple = ("proj", "v", "attn", "out")):
    key = ("prog", n_iters, debug, use_collective, stages)
    if key not in _CACHE:
        _CACHE[key] = _build_program(n_iters, debug=debug,
                                     use_collective=use_collective, stages=stages)
    return _CACHE[key]


def kernel(x, mask, Wq, Wk, Wv, Wo):
    nc = get_program()
    in_maps = _prep_in_maps(x, mask, Wq, Wk, Wv, Wo)
    res = run_bass_kernel_spmd(nc, in_maps, core_ids=list(range(NCORES)))
    return _assemble(res.results)


# revision 18
# speedup vs baseline: 1216.2640x; 1.5196x over previous
"""Trainium2 Bass kernel for 16-head causal attention (B=2, L=2048, D=1024).

Sharding (8 NeuronCores, one chip):
  - Head tensor-parallel: core c computes heads {2c, 2c+1} for BOTH batches.
    QKV projections are computed in transposed (d-major) layout directly:
    Q^T = Wq_pair @ x^T (x^T and W^T are host-prepped), so the S^T = K^T x Q
    matmul needs no on-device activation transposes. V is computed kv-major
    directly (x as the stationary operand), so no transposes there either.
  - Attention: S^T tiles [128 kv, 512 q], additive causal mask on diagonal
    blocks, un-stabilized exp on ScalarE (scale=1/8 folded into ACT), P*V with
    a ones-augmented V (softmax denominators fall out of the matmul).
  - One 8-way AllToAll redistributes ctx^T from head-sharded to
    (batch, L/4-slice)-sharded; each core then computes
    out^T = Wo @ ctx_full^T for its 512-row output slice.
  - Host: transpose/concat per-core out^T slices into the full output.

Precision: fp32/f32r matmuls are pathologically slow on this target
(~55us/instruction), so all matmuls run in fp16 (10 mantissa bits) with fp32
PSUM accumulation. All tensors here are comfortably inside fp16 range.
Measured ~5e-4 scale-relative output error vs the fp32 reference.
"""

import numpy as np

import concourse.bass as bass
import concourse.mybir as mybir
import concourse.tile as tile
from concourse import bacc
from concourse.bass_utils import run_bass_kernel_spmd

F32 = mybir.dt.float32
FP16 = mybir.dt.float16

B, L, D = 2, 2048, 1024
NCORES = 8
QC = 512   # q-chunk width
KB = 128   # kv-block width
NEG = -3.0e4  # additive mask; exp(scale*NEG) == 0 exactly

_CACHE: dict = {}


def _build_program(n_iters: int = 1, debug: bool = False, use_collective: bool = True,
                   stages: tuple = ("proj", "v", "attn", "out")):
    """Build the SPMD Bass program (same on all cores; per-core data differs)."""
    nc = bacc.Bacc(debug=debug)

    par = {}
    for nm, shape, dt in (
        ("xT", [B, D, L], FP16),
        ("wqT", [D, 128], FP16),
        ("wkT", [D, 128], FP16),
        ("wvT", [D, 128], FP16),
        ("woT", [D, D], FP16),
        ("maskadd", [4, 128, QC], F32),
    ):
        par[nm] = nc.declare_dram_parameter(nm, shape, dt, isOutput=False)
    outT = nc.declare_dram_parameter("outT", [D, QC], F32, isOutput=True)

    a2a_in = nc.dram_tensor("a2a_in", [NCORES * 128, QC], FP16)
    a2a_out = nc.dram_tensor("a2a_out", [NCORES * 128, QC], FP16)
    groups = [list(range(NCORES))]

    with tile.TileContext(nc) as tc:
        for _ in range(n_iters):
            _emit_iteration(nc, tc, par, outT, a2a_in, a2a_out, groups,
                            use_collective, stages)

    nc.compile()
    return nc


def _emit_iteration(nc, tc, par, outT, a2a_in, a2a_out, groups,
                    use_collective=True, stages=("proj", "v", "attn", "out")):
    with (
        tc.tile_pool(name="const", bufs=1) as const_p,
        tc.tile_pool(name="w", bufs=1) as w_p,
        tc.tile_pool(name="qk", bufs=2) as qk_p,
        tc.tile_pool(name="vt", bufs=32) as v_p,
        tc.tile_pool(name="ctx", bufs=2) as ctx_p,
        tc.tile_pool(name="msk", bufs=4) as msk_p,
        tc.tile_pool(name="p", bufs=4) as p_p,
        tc.tile_pool(name="r", bufs=2) as r_p,
        tc.tile_pool(name="bs", bufs=2) as bs_p,
        tc.tile_pool(name="ps", bufs=2, space="PSUM") as ps_p,
        tc.tile_pool(name="st", bufs=4, space="PSUM") as st_p,
        tc.tile_pool(name="cps", bufs=2, space="PSUM") as cps_p,
    ):
        # ---- constants ----
        ones_f = const_p.tile([128, 1], F32)
        nc.gpsimd.memset(ones_f[:], 1.0)
        ones64 = const_p.tile([1, 64], FP16)
        nc.vector.tensor_copy(ones64[:], ones_f[0:1, 0:1].to_broadcast([1, 64]))

        mask_sb = []
        for t in range(4):
            m = msk_p.tile([128, QC], F32, tag="mask")
            nc.sync.dma_start(m[:], par["maskadd"][t])
            mask_sb.append(m)

        # ---- weights (QKV) ----
        w_sb = {}
        for name in ("wqT", "wkT", "wvT"):
            w = w_p.tile([128, 8, 128], FP16, tag=name, name=name)
            nc.sync.dma_start(w[:], par[name].rearrange("(a p) m -> p a m", p=128))
            w_sb[name] = w

        ctx_sb = []  # per batch [128, 2048] fp16
        with tc.tile_pool(name="xt", bufs=8) as xt_p:
            for b in range(B):
                # ---- load x^T for this batch ----
                xt = []
                for a in range(8):
                    x_tile = xt_p.tile([128, L], FP16, tag="xt", name=f"xt{a}")
                    nc.sync.dma_start(
                        x_tile[:], par["xT"][b, 128 * a : 128 * (a + 1), :]
                    )
                    xt.append(x_tile)

                # ---- Q/K projections (d-major) ----
                def proj_dmajor(w, dst):
                    for n in range(4):
                        ns = slice(QC * n, QC * (n + 1))
                        ps = ps_p.tile([128, QC], F32, tag="ps", name="pps")
                        for a in range(8):
                            nc.tensor.matmul(
                                ps[:], lhsT=w[:, a, :], rhs=xt[a][:, ns],
                                start=(a == 0), stop=(a == 7),
                            )
                        nc.vector.tensor_copy(dst[:, ns], ps[:])

                qT = qk_p.tile([128, L], FP16, tag="qT", name="qT")
                kT = qk_p.tile([128, L], FP16, tag="kT", name="kT")
                if "proj" in stages:
                    proj_dmajor(w_sb["wqT"], qT)
                    proj_dmajor(w_sb["wkT"], kT)

                # ---- V projection (kv-major), ones column appended ----
                v_tiles = []
                for kb in range(16 if "v" in stages else 0):
                    ks = slice(KB * kb, KB * (kb + 1))
                    ps = ps_p.tile([128, 128], F32, tag="ps", name="vps")
                    for a in range(8):
                        nc.tensor.matmul(
                            ps[:], lhsT=xt[a][:, ks], rhs=w_sb["wvT"][:, a, :],
                            start=(a == 0), stop=(a == 7),
                        )
                    vt = v_p.tile([128, 130], FP16, tag="vt", name="vt")
                    for a in range(2):
                        hs = slice(65 * a, 65 * a + 64)
                        nc.vector.tensor_copy(vt[:, hs], ps[:, 64 * a : 64 * (a + 1)])
                        nc.vector.tensor_copy(
                            vt[:, 65 * a + 64 : 65 * a + 65], ones_f[:]
                        )
                    v_tiles.append(vt)

                # ---- attention ----
                ctxT = ctx_p.tile([128, L], FP16, tag="ctxT", name="ctxT")
                ctx_sb.append(ctxT)
                for jc in range(4 if "attn" in stages else 0):
                    qs = slice(QC * jc, QC * (jc + 1))
                    cps = [
                        cps_p.tile([65, QC], F32, tag="cps", name=f"cps{a}")
                        for a in range(2)
                    ]
                    nkb = 4 * jc + 4
                    for kb in range(nkb):
                        ks = slice(KB * kb, KB * (kb + 1))
                        p_t = []
                        for a in range(2):
                            da = slice(64 * a, 64 * (a + 1))
                            st = st_p.tile([128, QC], F32, tag="st", name="st")
                            nc.tensor.matmul(
                                st[:], lhsT=kT[da, ks], rhs=qT[da, qs],
                                start=True, stop=True,
                                tile_position=(64 * a, 0),
                            )
                            if kb >= 4 * jc:
                                w = 128 * (kb - 4 * jc + 1)
                                nc.vector.tensor_add(
                                    st[:, :w], st[:, :w], mask_sb[kb - 4 * jc][:, :w]
                                )
                            p = p_p.tile([128, QC], FP16, tag="p", name="p")
                            nc.scalar.activation(
                                p[:], st[:], mybir.ActivationFunctionType.Exp,
                                scale=0.125,
                            )
                            p_t.append(p)
                        for a in range(2):
                            nc.tensor.matmul(
                                cps[a][:],
                                lhsT=v_tiles[kb][:, 65 * a : 65 * a + 65],
                                rhs=p_t[a][:],
                                start=(kb == 0), stop=(kb == nkb - 1),
                            )
                    # normalize: ctx rows for each head *= 1/denominator
                    for a in range(2):
                        rf = r_p.tile([1, QC], F32, tag="rf", name="rf")
                        nc.vector.reciprocal(rf[:], cps[a][64:65, :])
                        rh = r_p.tile([1, QC], FP16, tag="rh", name="rh")
                        nc.vector.tensor_copy(rh[:], rf[:])
                        bc = ps_p.tile([64, QC], F32, tag="ps", name="bps")
                        nc.tensor.matmul(bc[:], lhsT=ones64[:], rhs=rh[:],
                                         start=True, stop=True)
                        bs = bs_p.tile([64, QC], F32, tag="bs", name="bs")
                        nc.vector.tensor_copy(bs[:], bc[:])
                        da = slice(64 * a, 64 * (a + 1))
                        nc.vector.tensor_mul(ctxT[da, qs], cps[a][0:64, :], bs[:])

        if "attn" not in stages or "out" not in stages:
            with tc.tile_pool(name="dummy", bufs=1) as d_p:
                dt_ = d_p.tile([128, QC], F32, tag="d", name="dummy_t")
                nc.vector.tensor_copy(dt_[:], mask_sb[0][:])
                for ob in range(8):
                    nc.sync.dma_start(outT[128 * ob : 128 * (ob + 1), :], dt_[:])
            return

        # ---- A2A staging: one DMA per batch ----
        a2a_in_v = a2a_in.rearrange("(j p) n -> p j n", p=128)
        for b in range(B):
            nc.sync.dma_start(
                a2a_in_v[:, 4 * b : 4 * (b + 1), :],
                ctx_sb[b][:].rearrange("p (t n) -> p t n", n=QC),
            )
        if use_collective:
            nc.gpsimd.collective_compute(
                "AllToAll",
                mybir.AluOpType.bypass,
                replica_groups=groups,
                ins=[a2a_in[:]],
                outs=[a2a_out[:]],
            )
        else:
            a2a_out = a2a_in

        # ---- out projection: out^T = Wo @ ctx_full^T ----
        with (
            tc.tile_pool(name="wo", bufs=1) as wo_p,
            tc.tile_pool(name="ao", bufs=8) as ao_p,
            tc.tile_pool(name="os", bufs=4) as os_p,
        ):
            wo_sb = wo_p.tile([128, 8, D], FP16, tag="wo", name="wo")
            nc.sync.dma_start(
                wo_sb[:], par["woT"].rearrange("(a p) m -> p a m", p=128)
            )
            ao = []
            for a in range(8):
                t = ao_p.tile([128, QC], FP16, tag="ao", name=f"ao{a}")
                nc.sync.dma_start(t[:], a2a_out[128 * a : 128 * (a + 1), :])
                ao.append(t)
            for ob in range(8):
                os_ = slice(128 * ob, 128 * (ob + 1))
                ps = ps_p.tile([128, QC], F32, tag="ps", name="ops")
                for a in range(8):
                    nc.tensor.matmul(
                        ps[:], lhsT=wo_sb[:, a, os_], rhs=ao[a][:],
                        start=(a == 0), stop=(a == 7),
                    )
                o_sb = os_p.tile([128, QC], F32, tag="os", name="osb")
                nc.vector.tensor_copy(o_sb[:], ps[:])
                nc.sync.dma_start(outT[os_, :], o_sb[:])


def _prep_in_maps(x, mask, Wq, Wk, Wv, Wo):
    x = np.asarray(x, dtype=np.float32)
    mask = np.asarray(mask)

    xT = np.ascontiguousarray(x.transpose(0, 2, 1).astype(np.float16))
    woT = np.ascontiguousarray(np.asarray(Wo, np.float32).T.astype(np.float16))

    m0 = mask[0]
    maskadd = np.stack(
        [
            np.where(m0[0:QC, KB * t : KB * (t + 1)].T, 0.0, NEG).astype(np.float32)
            for t in range(4)
        ]
    )

    in_maps = []
    for c in range(NCORES):
        rows = slice(128 * c, 128 * (c + 1))
        in_maps.append(
            {
                "xT": xT,
                "wqT": np.ascontiguousarray(
                    np.asarray(Wq, np.float32)[rows, :].T.astype(np.float16)
                ),
                "wkT": np.ascontiguousarray(
                    np.asarray(Wk, np.float32)[rows, :].T.astype(np.float16)
                ),
                "wvT": np.ascontiguousarray(
                    np.asarray(Wv, np.float32)[rows, :].T.astype(np.float16)
                ),
                "woT": woT,
                "maskadd": maskadd,
            }
        )
    return in_maps


def _assemble(results):
    out = np.empty((B, L, D), np.float32)
    for c in range(NCORES):
        outT = results[c]["outT"]
        out[c // 4, QC * (c % 4) : QC * (c % 4 + 1), :] = outT.T
    return out


def get_program(n_iters: int = 1, debug: bool = False, use_collective: bool = True,
                stages: tuple = ("proj", "v", "attn", "out")):
    key = ("prog", n_iters, debug, use_collective, stages)
    if key not in _CACHE:
        _CACHE[key] = _build_program(n_iters, debug=debug,
                                     use_collective=use_collective, stages=stages)
    return _CACHE[key]


def kernel(x, mask, Wq, Wk, Wv, Wo):
    nc = get_program()
    in_maps = _prep_in_maps(x, mask, Wq, Wk, Wv, Wo)
    res = run_bass_kernel_spmd(nc, in_maps, core_ids=list(range(NCORES)))
    return _assemble(res.results)


# revision 23
# speedup vs baseline: 1738.4931x; 1.4294x over previous
"""Trainium2 Bass kernel for 16-head causal attention (B=2, L=2048, D=1024).

Sharding (8 NeuronCores, one chip):
  - Head tensor-parallel: core c computes heads {2c, 2c+1} for BOTH batches.
    QKV projections are computed in transposed (d-major) layout directly:
    Q^T = Wq_pair @ x^T (x^T and W^T are host-prepped), so the S^T = K^T x Q
    matmul needs no on-device activation transposes. V is computed kv-major
    directly (x as the stationary operand), so no transposes there either.
  - Attention: S^T tiles [128 kv, 512 q], additive causal mask on diagonal
    blocks, un-stabilized exp on ScalarE (scale=1/8 folded into ACT), P*V with
    a ones-augmented V (softmax denominators fall out of the matmul).
  - One 8-way AllToAll redistributes ctx^T from head-sharded to
    (batch, L/4-slice)-sharded; each core then computes
    out^T = Wo @ ctx_full^T for its 512-row output slice.
  - Host: transpose/concat per-core out^T slices into the full output.

Precision: fp32/f32r matmuls are pathologically slow on this target
(~55us/instruction), so all matmuls run in fp16 (10 mantissa bits) with fp32
PSUM accumulation. All tensors here are comfortably inside fp16 range.
Measured ~5e-4 scale-relative output error vs the fp32 reference.
"""

import numpy as np

import concourse.bass as bass
import concourse.mybir as mybir
import concourse.tile as tile
from concourse import bacc
from concourse.bass_utils import run_bass_kernel_spmd

F32 = mybir.dt.float32
FP16 = mybir.dt.float16

B, L, D = 2, 2048, 1024
NCORES = 8
QC = 512   # q-chunk width
KB = 128   # kv-block width
NEG = -3.0e4  # additive mask; exp(scale*NEG) == 0 exactly

_CACHE: dict = {}


def _build_program(n_iters: int = 1, debug: bool = False, use_collective: bool = True,
                   stages: tuple = ("proj", "v", "attn", "out")):
    """Build the SPMD Bass program (same on all cores; per-core data differs)."""
    nc = bacc.Bacc(debug=debug)

    par = {}
    for nm, shape, dt in (
        ("xT", [B, D, L], FP16),
        ("wqT", [D, 128], FP16),
        ("wkT", [D, 128], FP16),
        ("wvT", [D, 128], FP16),
        ("woT", [D, D], FP16),
        ("maskadd", [4, 128, QC], F32),
    ):
        par[nm] = nc.declare_dram_parameter(nm, shape, dt, isOutput=False)
    outT = nc.declare_dram_parameter("outT", [D, QC], F32, isOutput=True)

    a2a_in = nc.dram_tensor("a2a_in", [NCORES * 128, QC], FP16)
    a2a_out = nc.dram_tensor("a2a_out", [NCORES * 128, QC], FP16)
    groups = [list(range(NCORES))]

    with tile.TileContext(nc) as tc:
        for _ in range(n_iters):
            _emit_iteration(nc, tc, par, outT, a2a_in, a2a_out, groups,
                            use_collective, stages)

    nc.compile()
    return nc


def _emit_iteration(nc, tc, par, outT, a2a_in, a2a_out, groups,
                    use_collective=True, stages=("proj", "v", "attn", "out")):
    with (
        tc.tile_pool(name="const", bufs=1) as const_p,
        tc.tile_pool(name="w", bufs=1) as w_p,
        tc.tile_pool(name="qk", bufs=2) as qk_p,
        tc.tile_pool(name="vt", bufs=32) as v_p,
        tc.tile_pool(name="ctx", bufs=2) as ctx_p,
        tc.tile_pool(name="msk", bufs=4) as msk_p,
        tc.tile_pool(name="p", bufs=4) as p_p,
        tc.tile_pool(name="r", bufs=2) as r_p,
        tc.tile_pool(name="bs", bufs=2) as bs_p,
        tc.tile_pool(name="ps", bufs=2, space="PSUM") as ps_p,
        tc.tile_pool(name="st", bufs=2, space="PSUM") as st_p,
        tc.tile_pool(name="cps", bufs=2, space="PSUM") as cps_p,
    ):
        # ---- constants ----
        ones_f = const_p.tile([128, 1], F32)
        nc.gpsimd.memset(ones_f[:], 1.0)
        ones64 = const_p.tile([1, 64], FP16)
        nc.vector.tensor_copy(ones64[:], ones_f[0:1, 0:1].to_broadcast([1, 64]))

        mask_sb = []
        for t in range(4):
            # duplicated for the two heads' side-by-side S^T layout
            m = msk_p.tile([128, 2, QC], F32, tag="mask", name="mask")
            nc.sync.dma_start(m[:, 0, :], par["maskadd"][t])
            nc.sync.dma_start(m[:, 1, :], par["maskadd"][t])
            mask_sb.append(m)

        # ---- weights (QKV) ----
        w_sb = {}
        for name in ("wqT", "wkT", "wvT"):
            w = w_p.tile([128, 8, 128], FP16, tag=name, name=name)
            nc.sync.dma_start(w[:], par[name].rearrange("(a p) m -> p a m", p=128))
            w_sb[name] = w

        ctx_sb = []  # per batch [128, 2048] fp16
        with tc.tile_pool(name="xt", bufs=8) as xt_p:
            for b in range(B):
                # ---- load x^T for this batch ----
                xt = []
                for a in range(8):
                    x_tile = xt_p.tile([128, L], FP16, tag="xt", name=f"xt{a}")
                    nc.sync.dma_start(
                        x_tile[:], par["xT"][b, 128 * a : 128 * (a + 1), :]
                    )
                    xt.append(x_tile)

                # ---- Q/K projections (d-major) ----
                def proj_dmajor(w, dst):
                    for n in range(4):
                        ns = slice(QC * n, QC * (n + 1))
                        ps = ps_p.tile([128, QC], F32, tag="ps", name="pps")
                        for a in range(8):
                            nc.tensor.matmul(
                                ps[:], lhsT=w[:, a, :], rhs=xt[a][:, ns],
                                start=(a == 0), stop=(a == 7),
                            )
                        nc.vector.tensor_copy(dst[:, ns], ps[:])

                qT = qk_p.tile([128, L], FP16, tag="qT", name="qT")
                kT = qk_p.tile([128, L], FP16, tag="kT", name="kT")
                if "proj" in stages:
                    proj_dmajor(w_sb["wqT"], qT)
                    proj_dmajor(w_sb["wkT"], kT)

                # ---- V projection (kv-major), ones column appended ----
                v_tiles = []
                for kb in range(16 if "v" in stages else 0):
                    ks = slice(KB * kb, KB * (kb + 1))
                    ps = ps_p.tile([128, 128], F32, tag="ps", name="vps")
                    for a in range(8):
                        nc.tensor.matmul(
                            ps[:], lhsT=xt[a][:, ks], rhs=w_sb["wvT"][:, a, :],
                            start=(a == 0), stop=(a == 7),
                        )
                    vt = v_p.tile([128, 130], FP16, tag="vt", name="vt")
                    for a in range(2):
                        hs = slice(65 * a, 65 * a + 64)
                        nc.vector.tensor_copy(vt[:, hs], ps[:, 64 * a : 64 * (a + 1)])
                        nc.vector.tensor_copy(
                            vt[:, 65 * a + 64 : 65 * a + 65], ones_f[:]
                        )
                    v_tiles.append(vt)

                # ---- attention ----
                ctxT = ctx_p.tile([128, L], FP16, tag="ctxT", name="ctxT")
                ctx_sb.append(ctxT)
                for jc in range(4 if "attn" in stages else 0):
                    qs = slice(QC * jc, QC * (jc + 1))
                    cps = [
                        cps_p.tile([65, QC], F32, tag="cps", name=f"cps{a}")
                        for a in range(2)
                    ]
                    nkb = 4 * jc + 4
                    for kb in range(nkb):
                        ks = slice(KB * kb, KB * (kb + 1))
                        # both heads' S^T side by side in one 2-bank PSUM tile
                        st = st_p.tile([128, 2, QC], F32, tag="st", name="st")
                        for a in range(2):
                            da = slice(64 * a, 64 * (a + 1))
                            nc.tensor.matmul(
                                st[:, a, :], lhsT=kT[da, ks], rhs=qT[da, qs],
                                start=True, stop=True,
                                tile_position=(64 * a, 0),
                            )
                        p = p_p.tile([128, 2, QC], FP16, tag="p", name="p")
                        if kb >= 4 * jc:
                            # diagonal block: columns < w0 are (partially)
                            # masked; columns >= w0 need no mask at all.
                            t = kb - 4 * jc
                            w = 128 * (t + 1)
                            w0 = 128 * t
                            nc.vector.tensor_add(
                                st[:, :, :w], st[:, :, :w], mask_sb[t][:, :, :w]
                            )
                            if w0 > 0:
                                # fully-masked prefix: exp == 0, skip the ACT
                                nc.gpsimd.memset(p[:, :, :w0], 0.0)
                            nc.scalar.activation(
                                p[:, :, w0:], st[:, :, w0:],
                                mybir.ActivationFunctionType.Exp, scale=0.125,
                            )
                        else:
                            nc.scalar.activation(
                                p[:], st[:], mybir.ActivationFunctionType.Exp,
                                scale=0.125,
                            )
                        for a in range(2):
                            nc.tensor.matmul(
                                cps[a][:],
                                lhsT=v_tiles[kb][:, 65 * a : 65 * a + 65],
                                rhs=p[:, a, :],
                                start=(kb == 0), stop=(kb == nkb - 1),
                            )
                    # normalize: ctx rows for each head *= 1/denominator
                    for a in range(2):
                        rf = r_p.tile([1, QC], F32, tag="rf", name="rf")
                        nc.vector.reciprocal(rf[:], cps[a][64:65, :])
                        rh = r_p.tile([1, QC], FP16, tag="rh", name="rh")
                        nc.vector.tensor_copy(rh[:], rf[:])
                        bc = ps_p.tile([64, QC], F32, tag="ps", name="bps")
                        nc.tensor.matmul(bc[:], lhsT=ones64[:], rhs=rh[:],
                                         start=True, stop=True)
                        bs = bs_p.tile([64, QC], F32, tag="bs", name="bs")
                        nc.vector.tensor_copy(bs[:], bc[:])
                        da = slice(64 * a, 64 * (a + 1))
                        nc.vector.tensor_mul(ctxT[da, qs], cps[a][0:64, :], bs[:])

        if "attn" not in stages or "out" not in stages:
            with tc.tile_pool(name="dummy", bufs=1) as d_p:
                dt_ = d_p.tile([128, QC], F32, tag="d", name="dummy_t")
                nc.vector.tensor_copy(dt_[:], mask_sb[0][:])
                for ob in range(8):
                    nc.sync.dma_start(outT[128 * ob : 128 * (ob + 1), :], dt_[:])
            return

        # ---- A2A staging: one DMA per batch ----
        a2a_in_v = a2a_in.rearrange("(j p) n -> p j n", p=128)
        for b in range(B):
            nc.sync.dma_start(
                a2a_in_v[:, 4 * b : 4 * (b + 1), :],
                ctx_sb[b][:].rearrange("p (t n) -> p t n", n=QC),
            )
        if use_collective:
            nc.gpsimd.collective_compute(
                "AllToAll",
                mybir.AluOpType.bypass,
                replica_groups=groups,
                ins=[a2a_in[:]],
                outs=[a2a_out[:]],
            )
        else:
            a2a_out = a2a_in

        # ---- out projection: out^T = Wo @ ctx_full^T ----
        with (
            tc.tile_pool(name="wo", bufs=1) as wo_p,
            tc.tile_pool(name="ao", bufs=8) as ao_p,
            tc.tile_pool(name="os", bufs=4) as os_p,
        ):
            wo_sb = wo_p.tile([128, 8, D], FP16, tag="wo", name="wo")
            nc.sync.dma_start(
                wo_sb[:], par["woT"].rearrange("(a p) m -> p a m", p=128)
            )
            ao = []
            for a in range(8):
                t = ao_p.tile([128, QC], FP16, tag="ao", name=f"ao{a}")
                nc.sync.dma_start(t[:], a2a_out[128 * a : 128 * (a + 1), :])
                ao.append(t)
            for ob in range(8):
                os_ = slice(128 * ob, 128 * (ob + 1))
                ps = ps_p.tile([128, QC], F32, tag="ps", name="ops")
                for a in range(8):
                    nc.tensor.matmul(
                        ps[:], lhsT=wo_sb[:, a, os_], rhs=ao[a][:],
                        start=(a == 0), stop=(a == 7),
                    )
                o_sb = os_p.tile([128, QC], F32, tag="os", name="osb")
                nc.vector.tensor_copy(o_sb[:], ps[:])
                nc.sync.dma_start(outT[os_, :], o_sb[:])


def _prep_in_maps(x, mask, Wq, Wk, Wv, Wo):
    x = np.asarray(x, dtype=np.float32)
    mask = np.asarray(mask)

    xT = np.ascontiguousarray(x.transpose(0, 2, 1).astype(np.float16))
    woT = np.ascontiguousarray(np.asarray(Wo, np.float32).T.astype(np.float16))

    m0 = mask[0]
    maskadd = np.stack(
        [
            np.where(m0[0:QC, KB * t : KB * (t + 1)].T, 0.0, NEG).astype(np.float32)
            for t in range(4)
        ]
    )

    in_maps = []
    for c in range(NCORES):
        rows = slice(128 * c, 128 * (c + 1))
        in_maps.append(
            {
                "xT": xT,
                "wqT": np.ascontiguousarray(
                    np.asarray(Wq, np.float32)[rows, :].T.astype(np.float16)
                ),
                "wkT": np.ascontiguousarray(
                    np.asarray(Wk, np.float32)[rows, :].T.astype(np.float16)
                ),
                "wvT": np.ascontiguousarray(
                    np.asarray(Wv, np.float32)[rows, :].T.astype(np.float16)
                ),
                "woT": woT,
                "maskadd": maskadd,
            }
        )
    return in_maps


def _assemble(results):
    out = np.empty((B, L, D), np.float32)
    for c in range(NCORES):
        outT = results[c]["outT"]
        out[c // 4, QC * (c % 4) : QC * (c % 4 + 1), :] = outT.T
    return out


def get_program(n_iters: int = 1, debug: bool = False, use_collective: bool = True,
                stages: tuple = ("proj", "v", "attn", "out")):
    key = ("prog", n_iters, debug, use_collective, stages)
    if key not in _CACHE:
        _CACHE[key] = _build_program(n_iters, debug=debug,
                                     use_collective=use_collective, stages=stages)
    return _CACHE[key]


def kernel(x, mask, Wq, Wk, Wv, Wo):
    nc = get_program()
    in_maps = _prep_in_maps(x, mask, Wq, Wk, Wv, Wo)
    res = run_bass_kernel_spmd(nc, in_maps, core_ids=list(range(NCORES)))
    return _assemble(res.results)
